# revision 27
# baseline (speedup 1.0000x reference)
"""Trainium2 Bass kernel for nn_MoELayer (moe_routing).

Expert-parallel across 8 NeuronCores, host-side routing:
  - host computes the gate (x@Wg + biases), top-2, sigmoid+normalize
    (33 MFLOP on 8192 tokens -- trivial), and builds per-expert dispatch
    lists. Core e receives ONLY the tokens routed to expert e (~2176
    padded slots instead of all 8192), already gathered and
    channel-transposed to [128, KC, NSLOT] bf16.
  - device (per core): GEMM1 [C->HID] -> exact GELU (+b1, ACT engine)
    -> GEMM2 [HID->C_OUT], all bf16 with fp32 PSUM accumulation;
    unscaled yT [C_OUT, NSLOT] DMA'd straight from PSUM to DRAM.
  - host combine: out[t] = g0*(y[e0,p0]+b2[e0]) + g1*(y[e1,p1]+b2[e1]).

This is the top-2-sparse compute (4x fewer MACs than the dense
comb-weighted formulation) with zero data-dependent DMA on device.
"""

import os
import sys

sys.path.insert(0, "/opt/trn_rl_repo")
os.environ.setdefault("JAX_PLATFORMS", "")
os.environ.setdefault("NEURON_RT_RESET_CORES", "1")

import numpy as np
import ml_dtypes

B, M, H, W, C = 2, 4, 32, 32, 256
E, TOPK, HID, C_OUT = 8, 2, 512, 256
T = B * M * H * W          # 8192 tokens
NCORES = 8
P = 128
KC = C // P                # 2 contraction subtiles over C
KH = HID // P              # 4 contraction subtiles over HID
CT = C_OUT // P            # 2 output-partition tiles over C_OUT
CH = 512                   # slot chunk (one PSUM bank of fp32)

_BUILD_CACHE = {}


def _chunks(n):
    """Slot chunks: tiny then small first chunks (pipeline fill: first
    GEMM1 starts as soon as 64 slots have landed), ~512 middles, small
    tail chunk (pipeline drain)."""
    cs = []
    s = 0
    for w in (64, 256):
        if n - s <= 0:
            break
        cs.append((s, min(s + w, n)))
        s = min(s + w, n)
    while n - s > 768:
        cs.append((s, s + CH))
        s += CH
    rem = n - s
    if rem > 512:
        cs.append((s, n - 256))
        cs.append((n - 256, n))
    elif rem > 0:
        cs.append((s, n))
    return cs


def _build(nslot):
    import concourse.bacc as bacc
    import concourse.mybir as mybir
    from concourse.tile import TileContext

    dt = mybir.dt
    AF = mybir.ActivationFunctionType

    chunks = _chunks(nslot)
    ncnk = len(chunks)
    # xg DMA pieces = compute chunks: fine-grained so chunk j+1 is never
    # stuck behind a big multi-chunk transfer
    pieces = list(chunks)

    nc = bacc.Bacc("TRN2", target_bir_lowering=False)

    xg_d = nc.dram_tensor("xg", [P, KC, nslot], dt.bfloat16, kind="ExternalInput")
    w1_d = nc.dram_tensor("w1", [P, KC, HID], dt.bfloat16, kind="ExternalInput")
    w2_d = nc.dram_tensor("w2", [P, KH, C_OUT], dt.bfloat16, kind="ExternalInput")
    b1_d = nc.dram_tensor("b1", [P, KH], dt.float32, kind="ExternalInput")
    y_d = nc.dram_tensor("y", [CT, P, nslot], dt.bfloat16, kind="ExternalOutput")
    y_r = y_d.rearrange("c p w -> p c w")

    with TileContext(nc) as tc:
        with (
            tc.tile_pool(name="const", bufs=1) as cpool,
            tc.tile_pool(name="hbuf", bufs=3) as hpool,
            tc.tile_pool(name="psh", bufs=5, space="PSUM") as psh,
            tc.tile_pool(name="pswm", bufs=1, space="PSUM") as pswm,
            tc.tile_pool(name="psy", bufs=2, space="PSUM") as psy,
        ):
            # ---- warmup: keep PE busy + preload GELU table during DMA-in.
            # One-partition memset is the cheapest possible producer; the
            # warmup matmuls contract over that single partition and write a
            # PSUM tile nothing reads.
            wm_sb = cpool.tile([1, 192], dt.bfloat16)
            nc.gpsimd.memset(wm_sb[:], 0.0)
            wmg_sb = cpool.tile([1, 8], dt.float32)
            nc.scalar.activation(wmg_sb[:], wm_sb[:, 0:8], AF.Gelu)
            ps_wm = pswm.tile([P, 64], dt.float32, tag="wm")
            for _ in range(56):
                nc.tensor.matmul(
                    ps_wm[:], lhsT=wm_sb[:, 0:P], rhs=wm_sb[:, 0:64],
                    start=True, stop=True,
                )

            # ---- inputs into SBUF (xg in pieces so compute starts early) ----
            w1_sb = cpool.tile([P, KC, HID], dt.bfloat16)
            xg_sb = cpool.tile([P, KC, nslot], dt.bfloat16)
            b1_sb = cpool.tile([P, KH], dt.float32)
            w2_sb = cpool.tile([P, KH, C_OUT], dt.bfloat16)

            nc.sync.dma_start(w1_sb[:], w1_d[:])
            nc.sync.dma_start(xg_sb[:, :, pieces[0][0]:pieces[0][1]],
                              xg_d[:, :, pieces[0][0]:pieces[0][1]])
            nc.sync.dma_start(b1_sb[:], b1_d[:])
            for s0, s1 in pieces[1:2]:
                nc.sync.dma_start(xg_sb[:, :, s0:s1], xg_d[:, :, s0:s1])
            nc.sync.dma_start(w2_sb[:], w2_d[:])
            for s0, s1 in pieces[2:]:
                nc.sync.dma_start(xg_sb[:, :, s0:s1], xg_d[:, :, s0:s1])
            del pieces

            hts = [None] * ncnk

            def g1(j):
                s0, s1 = chunks[j]
                cw = s1 - s0
                hT = hpool.tile([P, KH, CH], dt.bfloat16, tag="hT")
                for hc in range(KH):
                    ps_h = psh.tile([P, CH], dt.float32, tag="h")
                    for k in range(KC):
                        nc.tensor.matmul(
                            ps_h[:, :cw],
                            lhsT=w1_sb[:, k, hc * P:(hc + 1) * P],
                            rhs=xg_sb[:, k, s0:s1],
                            start=(k == 0),
                            stop=(k == KC - 1),
                        )
                    nc.scalar.activation(
                        hT[:, hc, :cw], ps_h[:, :cw], AF.Gelu,
                        bias=b1_sb[:, hc:hc + 1],
                    )
                hts[j] = hT

            def g2(j):
                s0, s1 = chunks[j]
                cw = s1 - s0
                last = j == ncnk - 1
                hT = hts[j]
                ysb = hpool.tile([P, CT, CH], dt.bfloat16, tag="ysb")
                for ct in range(CT):
                    ps_y = psy.tile([P, CH], dt.float32, tag="y")
                    for hc in range(KH):
                        nc.tensor.matmul(
                            ps_y[:, :cw],
                            lhsT=w2_sb[:, hc, ct * P:(ct + 1) * P],
                            rhs=hT[:, hc, :cw],
                            start=(hc == 0),
                            stop=(hc == KH - 1),
                        )
                    nc.vector.tensor_copy(out=ysb[:, ct, :cw], in_=ps_y[:, :cw])
                    if last:
                        # tail: ship halves on separate DGE queues so the
                        # ct=0 DMA generation overlaps ct=1 compute+copy
                        eng = nc.scalar if ct == 0 else nc.sync
                        eng.dma_start(y_d[ct, :, s0:s1], ysb[:, ct, :cw])
                if not last:
                    nc.sync.dma_start(y_r[:, :, s0:s1], ysb[:, :, :cw])

            # software pipeline: PE stays one chunk ahead of the GELU->GEMM2
            g1(0)
            for j in range(1, ncnk):
                g1(j)
                g2(j - 1)
            g2(ncnk - 1)

    nc.compile()
    return nc


def _get_nc(nslot):
    if nslot not in _BUILD_CACHE:
        _BUILD_CACHE[nslot] = _build(nslot)
    return _BUILD_CACHE[nslot]


def _route(inputs):
    """Host gate: top-2 routing, gate weights, per-expert dispatch."""
    x = np.asarray(inputs["x"], dtype=np.float32).reshape(T, C)
    Wg = np.asarray(inputs["Wg"], dtype=np.float32)
    bg = np.asarray(inputs["bg"], dtype=np.float32)
    eb = np.asarray(inputs["expert_bias"], dtype=np.float32)

    logits = x @ Wg + bg + eb                          # [T, E]
    top2 = np.argsort(-logits, axis=1, kind="stable")[:, :TOPK]   # [T, 2]
    vals = np.take_along_axis(logits, top2, axis=1)
    probs = 1.0 / (1.0 + np.exp(-vals))
    g = probs / probs.sum(axis=1, keepdims=True)       # [T, 2]

    toks, pos_in_expert = [], np.zeros((E, T), dtype=np.int64)
    cnts = np.zeros(E, dtype=np.int64)
    sel = (top2[:, 0] == np.arange(E)[:, None]) | (top2[:, 1] == np.arange(E)[:, None])
    for e in range(E):
        tok_e = np.nonzero(sel[e])[0]
        cnts[e] = len(tok_e)
        pos_in_expert[e, tok_e] = np.arange(len(tok_e))
        toks.append(tok_e)
    return x, top2, g, toks, cnts, pos_in_expert


def _stage(x, inputs, toks, nslot):
    W1 = np.asarray(inputs["W1"], dtype=np.float32)
    b1 = np.asarray(inputs["b1"], dtype=np.float32)
    W2 = np.asarray(inputs["W2"], dtype=np.float32)

    in_maps = []
    for e in range(NCORES):
        xg = np.zeros((nslot, C), dtype=np.float32)
        xg[: len(toks[e])] = x[toks[e]]
        xgT = np.ascontiguousarray(
            xg.T.reshape(KC, P, nslot).transpose(1, 0, 2)
        ).astype(ml_dtypes.bfloat16)
        w1s = np.ascontiguousarray(
            W1[e].reshape(KC, P, HID).transpose(1, 0, 2)
        ).astype(ml_dtypes.bfloat16)
        w2s = np.ascontiguousarray(
            W2[e].reshape(KH, P, C_OUT).transpose(1, 0, 2)
        ).astype(ml_dtypes.bfloat16)
        b1s = np.ascontiguousarray(b1[e].reshape(KH, P).T)
        in_maps.append({"xg": xgT, "w1": w1s, "w2": w2s, "b1": b1s})
    return in_maps


def kernel(**inputs):
    from concourse.bass_utils import run_bass_kernel_spmd

    x, top2, g, toks, cnts, pos = _route(inputs)
    nslot = max(CH, int(-(-cnts.max() // 8) * 8))
    nc = _get_nc(nslot)
    in_maps = _stage(x, inputs, toks, nslot)
    res = run_bass_kernel_spmd(nc, in_maps, core_ids=list(range(NCORES)))

    # y[e] : [CT, P, nslot] -> [C_OUT, nslot]
    Y = np.stack(
        [np.asarray(res.results[e]["y"], dtype=np.float32).reshape(C_OUT, nslot)
         for e in range(NCORES)]
    )
    b2 = np.asarray(inputs["b2"], dtype=np.float32)
    tok_idx = np.arange(T)
    e0, e1 = top2[:, 0], top2[:, 1]
    p0 = pos[e0, tok_idx]
    p1 = pos[e1, tok_idx]
    out = (
        g[:, 0:1] * (Y[e0, :, p0] + b2[e0])
        + g[:, 1:2] * (Y[e1, :, p1] + b2[e1])
    )
    return out.reshape(B, M, H, W, C_OUT).astype(np.float32)


# revision 31
# speedup vs baseline: 1.0353x; 1.0353x over previous
"""Trainium2 Bass kernel for nn_MoELayer (moe_routing).

Expert-parallel across 8 NeuronCores, host-side routing:
  - host computes the gate (x@Wg + biases), top-2, sigmoid+normalize
    (33 MFLOP on 8192 tokens -- trivial), and builds per-expert dispatch
    lists. Core e receives ONLY the tokens routed to expert e (~2176
    padded slots instead of all 8192), already gathered and
    channel-transposed to [128, KC, NSLOT] bf16.
  - device (per core): GEMM1 [C->HID] -> exact GELU (+b1, ACT engine)
    -> GEMM2 [HID->C_OUT], all bf16 with fp32 PSUM accumulation;
    unscaled yT [C_OUT, NSLOT] DMA'd straight from PSUM to DRAM.
  - host combine: out[t] = g0*(y[e0,p0]+b2[e0]) + g1*(y[e1,p1]+b2[e1]).

This is the top-2-sparse compute (4x fewer MACs than the dense
comb-weighted formulation) with zero data-dependent DMA on device.
"""

import os
import sys

sys.path.insert(0, "/opt/trn_rl_repo")
os.environ.setdefault("JAX_PLATFORMS", "")
os.environ.setdefault("NEURON_RT_RESET_CORES", "1")

import numpy as np
import ml_dtypes

B, M, H, W, C = 2, 4, 32, 32, 256
E, TOPK, HID, C_OUT = 8, 2, 512, 256
T = B * M * H * W          # 8192 tokens
NCORES = 8
P = 128
KC = C // P                # 2 contraction subtiles over C
KH = HID // P              # 4 contraction subtiles over HID
CT = C_OUT // P            # 2 output-partition tiles over C_OUT
CH = 512                   # slot chunk (one PSUM bank of fp32)

_BUILD_CACHE = {}


WARM_BIG = 48      # warmup matmuls at free=64
WARM_SMALL = 0     # fine-grained warmup matmuls at free=16
LEAD = 1           # software-pipeline lead (chunks GEMM1 runs ahead of GEMM2)


def _chunks(n):
    """Slot chunks: small first chunk (pipeline fill), ~512 middles, small
    tail chunk (pipeline drain). Widths multiples of 8 and >=256 so DMA
    inner runs stay >=512B."""
    cs = []
    s = 0
    first = min(256, n)
    cs.append((0, first))
    s = first
    while n - s > 768:
        cs.append((s, s + CH))
        s += CH
    rem = n - s
    if rem > 512:
        cs.append((s, n - 256))
        cs.append((n - 256, n))
    elif rem > 0:
        cs.append((s, n))
    return cs


def _build(nslot):
    import concourse.bacc as bacc
    import concourse.mybir as mybir
    from concourse.tile import TileContext

    dt = mybir.dt
    AF = mybir.ActivationFunctionType

    chunks = _chunks(nslot)
    ncnk = len(chunks)
    # xg DMA pieces = compute chunks: fine-grained so chunk j+1 is never
    # stuck behind a big multi-chunk transfer
    pieces = list(chunks)

    nc = bacc.Bacc("TRN2", target_bir_lowering=False)

    xg_d = nc.dram_tensor("xg", [P, KC, nslot], dt.bfloat16, kind="ExternalInput")
    w1_d = nc.dram_tensor("w1", [P, KC, HID], dt.bfloat16, kind="ExternalInput")
    w2_d = nc.dram_tensor("w2", [P, KH, C_OUT], dt.bfloat16, kind="ExternalInput")
    b1_d = nc.dram_tensor("b1", [P, KH], dt.float32, kind="ExternalInput")
    y_d = nc.dram_tensor("y", [CT, P, nslot], dt.bfloat16, kind="ExternalOutput")
    y_r = y_d.rearrange("c p w -> p c w")

    with TileContext(nc) as tc:
        with (
            tc.tile_pool(name="const", bufs=1) as cpool,
            tc.tile_pool(name="hbuf", bufs=3) as hpool,
            tc.tile_pool(name="psh", bufs=5, space="PSUM") as psh,
            tc.tile_pool(name="pswm", bufs=1, space="PSUM") as pswm,
            tc.tile_pool(name="psy", bufs=2, space="PSUM") as psy,
        ):
            # ---- warmup: keep PE busy + preload GELU table during DMA-in.
            # One-partition memset is the cheapest possible producer; the
            # warmup matmuls contract over that single partition and write a
            # PSUM tile nothing reads.
            wm_sb = cpool.tile([1, 192], dt.bfloat16)
            nc.gpsimd.memset(wm_sb[:], 0.0)
            wmg_sb = cpool.tile([1, 8], dt.float32)
            nc.scalar.activation(wmg_sb[:], wm_sb[:, 0:8], AF.Gelu)
            ps_wm = pswm.tile([P, 64], dt.float32, tag="wm")
            for _ in range(WARM_BIG):
                nc.tensor.matmul(
                    ps_wm[:], lhsT=wm_sb[:, 0:P], rhs=wm_sb[:, 0:64],
                    start=True, stop=True,
                )
            for _ in range(WARM_SMALL):
                nc.tensor.matmul(
                    ps_wm[:, 0:16], lhsT=wm_sb[:, 0:P], rhs=wm_sb[:, 0:16],
                    start=True, stop=True,
                )

            # ---- inputs into SBUF (xg in pieces so compute starts early) ----
            w1_sb = cpool.tile([P, KC, HID], dt.bfloat16)
            xg_sb = cpool.tile([P, KC, nslot], dt.bfloat16)
            b1_sb = cpool.tile([P, KH], dt.float32)
            w2_sb = cpool.tile([P, KH, C_OUT], dt.bfloat16)

            nc.sync.dma_start(w1_sb[:], w1_d[:])
            nc.sync.dma_start(xg_sb[:, :, pieces[0][0]:pieces[0][1]],
                              xg_d[:, :, pieces[0][0]:pieces[0][1]])
            nc.sync.dma_start(b1_sb[:], b1_d[:])
            for s0, s1 in pieces[1:2]:
                nc.sync.dma_start(xg_sb[:, :, s0:s1], xg_d[:, :, s0:s1])
            nc.sync.dma_start(w2_sb[:], w2_d[:])
            for s0, s1 in pieces[2:]:
                nc.sync.dma_start(xg_sb[:, :, s0:s1], xg_d[:, :, s0:s1])
            del pieces

            hts = [None] * ncnk

            def g1(j):
                s0, s1 = chunks[j]
                cw = s1 - s0
                hT = hpool.tile([P, KH, CH], dt.bfloat16, tag="hT")
                for hc in range(KH):
                    ps_h = psh.tile([P, CH], dt.float32, tag="h")
                    for k in range(KC):
                        nc.tensor.matmul(
                            ps_h[:, :cw],
                            lhsT=w1_sb[:, k, hc * P:(hc + 1) * P],
                            rhs=xg_sb[:, k, s0:s1],
                            start=(k == 0),
                            stop=(k == KC - 1),
                        )
                    nc.scalar.activation(
                        hT[:, hc, :cw], ps_h[:, :cw], AF.Gelu,
                        bias=b1_sb[:, hc:hc + 1],
                    )
                hts[j] = hT

            def g2(j):
                s0, s1 = chunks[j]
                cw = s1 - s0
                last = j == ncnk - 1
                hT = hts[j]
                ysb = hpool.tile([P, CT, CH], dt.bfloat16, tag="ysb")
                for ct in range(CT):
                    ps_y = psy.tile([P, CH], dt.float32, tag="y")
                    for hc in range(KH):
                        nc.tensor.matmul(
                            ps_y[:, :cw],
                            lhsT=w2_sb[:, hc, ct * P:(ct + 1) * P],
                            rhs=hT[:, hc, :cw],
                            start=(hc == 0),
                            stop=(hc == KH - 1),
                        )
                    nc.vector.tensor_copy(out=ysb[:, ct, :cw], in_=ps_y[:, :cw])
                    if last:
                        # tail: ship halves on separate DGE queues so the
                        # ct=0 DMA generation overlaps ct=1 compute+copy
                        eng = nc.scalar if ct == 0 else nc.sync
                        eng.dma_start(y_d[ct, :, s0:s1], ysb[:, ct, :cw])
                if not last:
                    nc.sync.dma_start(y_r[:, :, s0:s1], ysb[:, :, :cw])

            # software pipeline: PE stays LEAD chunks ahead of GELU->GEMM2
            lead = min(LEAD, ncnk - 1)
            for j in range(lead):
                g1(j)
            for j in range(lead, ncnk):
                g1(j)
                g2(j - lead)
            for j in range(ncnk - lead, ncnk):
                g2(j)

    nc.compile()
    return nc


def _get_nc(nslot):
    if nslot not in _BUILD_CACHE:
        _BUILD_CACHE[nslot] = _build(nslot)
    return _BUILD_CACHE[nslot]


def _route(inputs):
    """Host gate: top-2 routing, gate weights, per-expert dispatch."""
    x = np.asarray(inputs["x"], dtype=np.float32).reshape(T, C)
    Wg = np.asarray(inputs["Wg"], dtype=np.float32)
    bg = np.asarray(inputs["bg"], dtype=np.float32)
    eb = np.asarray(inputs["expert_bias"], dtype=np.float32)

    logits = x @ Wg + bg + eb                          # [T, E]
    top2 = np.argsort(-logits, axis=1, kind="stable")[:, :TOPK]   # [T, 2]
    vals = np.take_along_axis(logits, top2, axis=1)
    probs = 1.0 / (1.0 + np.exp(-vals))
    g = probs / probs.sum(axis=1, keepdims=True)       # [T, 2]

    toks, pos_in_expert = [], np.zeros((E, T), dtype=np.int64)
    cnts = np.zeros(E, dtype=np.int64)
    sel = (top2[:, 0] == np.arange(E)[:, None]) | (top2[:, 1] == np.arange(E)[:, None])
    for e in range(E):
        tok_e = np.nonzero(sel[e])[0]
        cnts[e] = len(tok_e)
        pos_in_expert[e, tok_e] = np.arange(len(tok_e))
        toks.append(tok_e)
    return x, top2, g, toks, cnts, pos_in_expert


def _stage(x, inputs, toks, nslot):
    W1 = np.asarray(inputs["W1"], dtype=np.float32)
    b1 = np.asarray(inputs["b1"], dtype=np.float32)
    W2 = np.asarray(inputs["W2"], dtype=np.float32)

    in_maps = []
    for e in range(NCORES):
        xg = np.zeros((nslot, C), dtype=np.float32)
        xg[: len(toks[e])] = x[toks[e]]
        xgT = np.ascontiguousarray(
            xg.T.reshape(KC, P, nslot).transpose(1, 0, 2)
        ).astype(ml_dtypes.bfloat16)
        w1s = np.ascontiguousarray(
            W1[e].reshape(KC, P, HID).transpose(1, 0, 2)
        ).astype(ml_dtypes.bfloat16)
        w2s = np.ascontiguousarray(
            W2[e].reshape(KH, P, C_OUT).transpose(1, 0, 2)
        ).astype(ml_dtypes.bfloat16)
        b1s = np.ascontiguousarray(b1[e].reshape(KH, P).T)
        in_maps.append({"xg": xgT, "w1": w1s, "w2": w2s, "b1": b1s})
    return in_maps


def kernel(**inputs):
    from concourse.bass_utils import run_bass_kernel_spmd

    x, top2, g, toks, cnts, pos = _route(inputs)
    nslot = max(CH, int(-(-cnts.max() // 8) * 8))
    nc = _get_nc(nslot)
    in_maps = _stage(x, inputs, toks, nslot)
    res = run_bass_kernel_spmd(nc, in_maps, core_ids=list(range(NCORES)))

    # y[e] : [CT, P, nslot] -> [C_OUT, nslot]
    Y = np.stack(
        [np.asarray(res.results[e]["y"], dtype=np.float32).reshape(C_OUT, nslot)
         for e in range(NCORES)]
    )
    b2 = np.asarray(inputs["b2"], dtype=np.float32)
    tok_idx = np.arange(T)
    e0, e1 = top2[:, 0], top2[:, 1]
    p0 = pos[e0, tok_idx]
    p1 = pos[e1, tok_idx]
    out = (
        g[:, 0:1] * (Y[e0, :, p0] + b2[e0])
        + g[:, 1:2] * (Y[e1, :, p1] + b2[e1])
    )
    return out.reshape(B, M, H, W, C_OUT).astype(np.float32)


# revision 34
# speedup vs baseline: 1.0415x; 1.0061x over previous
"""Trainium2 Bass kernel for nn_MoELayer (moe_routing).

Expert-parallel across 8 NeuronCores, host-side routing:
  - host computes the gate (x@Wg + biases), top-2, sigmoid+normalize
    (33 MFLOP on 8192 tokens -- trivial), and builds per-expert dispatch
    lists. Core e receives ONLY the tokens routed to expert e (~2176
    padded slots instead of all 8192), already gathered and
    channel-transposed to [128, KC, NSLOT] bf16.
  - device (per core): GEMM1 [C->HID] -> exact GELU (+b1, ACT engine)
    -> GEMM2 [HID->C_OUT], all bf16 with fp32 PSUM accumulation;
    unscaled yT [C_OUT, NSLOT] DMA'd straight from PSUM to DRAM.
  - host combine: out[t] = g0*(y[e0,p0]+b2[e0]) + g1*(y[e1,p1]+b2[e1]).

This is the top-2-sparse compute (4x fewer MACs than the dense
comb-weighted formulation) with zero data-dependent DMA on device.
"""

import os
import sys

sys.path.insert(0, "/opt/trn_rl_repo")
os.environ.setdefault("JAX_PLATFORMS", "")
os.environ.setdefault("NEURON_RT_RESET_CORES", "1")

import numpy as np
import ml_dtypes

B, M, H, W, C = 2, 4, 32, 32, 256
E, TOPK, HID, C_OUT = 8, 2, 512, 256
T = B * M * H * W          # 8192 tokens
NCORES = 8
P = 128
KC = C // P                # 2 contraction subtiles over C
KH = HID // P              # 4 contraction subtiles over HID
CT = C_OUT // P            # 2 output-partition tiles over C_OUT
CH = 512                   # slot chunk (one PSUM bank of fp32)

_BUILD_CACHE = {}


WARM_BIG = 56      # warmup matmuls at free=64
WARM_SMALL = 0     # fine-grained warmup matmuls at free=16
LEAD = 1           # software-pipeline lead (chunks GEMM1 runs ahead of GEMM2)
FILL2 = 320        # width of the second chunk (pipeline-fill tuning)
FIRSTW = 256       # width of the first chunk
TAILW = 256        # width of the last chunk


def _chunks(n):
    """Slot chunks: small first chunks (pipeline fill), ~512 middles, small
    tail chunk (pipeline drain). Widths multiples of 8 and >=256 so DMA
    inner runs stay >=512B."""
    cs = []
    s = 0
    first = min(FIRSTW, n)
    cs.append((0, first))
    s = first
    if n - s > FILL2 + 768:
        cs.append((s, s + FILL2))
        s += FILL2
    while n - s > 768:
        cs.append((s, s + CH))
        s += CH
    rem = n - s   # 0 < rem <= 768 here
    if rem > CH or rem > TAILW + 256:
        cs.append((s, n - TAILW))
        cs.append((n - TAILW, n))
    elif rem > 0:
        cs.append((s, n))
    assert all(0 < b - a <= CH for a, b in cs)
    return cs


def _build(nslot):
    import concourse.bacc as bacc
    import concourse.mybir as mybir
    from concourse.tile import TileContext

    dt = mybir.dt
    AF = mybir.ActivationFunctionType

    chunks = _chunks(nslot)
    ncnk = len(chunks)
    # xg DMA pieces = compute chunks: fine-grained so chunk j+1 is never
    # stuck behind a big multi-chunk transfer
    pieces = list(chunks)

    nc = bacc.Bacc("TRN2", target_bir_lowering=False)

    xg_d = nc.dram_tensor("xg", [P, KC, nslot], dt.bfloat16, kind="ExternalInput")
    w1_d = nc.dram_tensor("w1", [P, KC, HID], dt.bfloat16, kind="ExternalInput")
    w2_d = nc.dram_tensor("w2", [P, KH, C_OUT], dt.bfloat16, kind="ExternalInput")
    b1_d = nc.dram_tensor("b1", [P, KH], dt.float32, kind="ExternalInput")
    y_d = nc.dram_tensor("y", [CT, P, nslot], dt.bfloat16, kind="ExternalOutput")
    y_r = y_d.rearrange("c p w -> p c w")

    with TileContext(nc) as tc:
        with (
            tc.tile_pool(name="const", bufs=1) as cpool,
            tc.tile_pool(name="hbuf", bufs=3) as hpool,
            tc.tile_pool(name="psh", bufs=5, space="PSUM") as psh,
            tc.tile_pool(name="pswm", bufs=1, space="PSUM") as pswm,
            tc.tile_pool(name="psy", bufs=2, space="PSUM") as psy,
        ):
            # ---- warmup: keep PE busy + preload GELU table during DMA-in.
            # One-partition memset is the cheapest possible producer; the
            # warmup matmuls contract over that single partition and write a
            # PSUM tile nothing reads.
            wm_sb = cpool.tile([1, 192], dt.bfloat16)
            nc.gpsimd.memset(wm_sb[:], 0.0)
            wmg_sb = cpool.tile([1, 8], dt.float32)
            nc.scalar.activation(wmg_sb[:], wm_sb[:, 0:8], AF.Gelu)
            ps_wm = pswm.tile([P, 64], dt.float32, tag="wm")
            for _ in range(WARM_BIG):
                nc.tensor.matmul(
                    ps_wm[:], lhsT=wm_sb[:, 0:P], rhs=wm_sb[:, 0:64],
                    start=True, stop=True,
                )
            for _ in range(WARM_SMALL):
                nc.tensor.matmul(
                    ps_wm[:, 0:16], lhsT=wm_sb[:, 0:P], rhs=wm_sb[:, 0:16],
                    start=True, stop=True,
                )

            # ---- inputs into SBUF (xg in pieces so compute starts early) ----
            w1_sb = cpool.tile([P, KC, HID], dt.bfloat16)
            xg_sb = cpool.tile([P, KC, nslot], dt.bfloat16)
            b1_sb = cpool.tile([P, KH], dt.float32)
            w2_sb = cpool.tile([P, KH, C_OUT], dt.bfloat16)

            nc.sync.dma_start(w1_sb[:], w1_d[:])
            nc.sync.dma_start(xg_sb[:, :, pieces[0][0]:pieces[0][1]],
                              xg_d[:, :, pieces[0][0]:pieces[0][1]])
            nc.sync.dma_start(b1_sb[:], b1_d[:])
            for s0, s1 in pieces[1:2]:
                nc.sync.dma_start(xg_sb[:, :, s0:s1], xg_d[:, :, s0:s1])
            nc.sync.dma_start(w2_sb[:], w2_d[:])
            for s0, s1 in pieces[2:]:
                nc.sync.dma_start(xg_sb[:, :, s0:s1], xg_d[:, :, s0:s1])
            del pieces

            hts = [None] * ncnk

            def g1(j):
                s0, s1 = chunks[j]
                cw = s1 - s0
                hT = hpool.tile([P, KH, CH], dt.bfloat16, tag="hT")
                for hc in range(KH):
                    ps_h = psh.tile([P, CH], dt.float32, tag="h")
                    for k in range(KC):
                        nc.tensor.matmul(
                            ps_h[:, :cw],
                            lhsT=w1_sb[:, k, hc * P:(hc + 1) * P],
                            rhs=xg_sb[:, k, s0:s1],
                            start=(k == 0),
                            stop=(k == KC - 1),
                        )
                    nc.scalar.activation(
                        hT[:, hc, :cw], ps_h[:, :cw], AF.Gelu,
                        bias=b1_sb[:, hc:hc + 1],
                    )
                hts[j] = hT

            def g2(j):
                s0, s1 = chunks[j]
                cw = s1 - s0
                last = j == ncnk - 1
                hT = hts[j]
                ysb = hpool.tile([P, CT, CH], dt.bfloat16, tag="ysb")
                for ct in range(CT):
                    ps_y = psy.tile([P, CH], dt.float32, tag="y")
                    for hc in range(KH):
                        nc.tensor.matmul(
                            ps_y[:, :cw],
                            lhsT=w2_sb[:, hc, ct * P:(ct + 1) * P],
                            rhs=hT[:, hc, :cw],
                            start=(hc == 0),
                            stop=(hc == KH - 1),
                        )
                    nc.vector.tensor_copy(out=ysb[:, ct, :cw], in_=ps_y[:, :cw])
                    if last:
                        # tail: ship halves on separate DGE queues so the
                        # ct=0 DMA generation overlaps ct=1 compute+copy
                        eng = nc.scalar if ct == 0 else nc.sync
                        eng.dma_start(y_d[ct, :, s0:s1], ysb[:, ct, :cw])
                if not last:
                    nc.sync.dma_start(y_r[:, :, s0:s1], ysb[:, :, :cw])

            # software pipeline: PE stays LEAD chunks ahead of GELU->GEMM2
            lead = min(LEAD, ncnk - 1)
            for j in range(lead):
                g1(j)
            for j in range(lead, ncnk):
                g1(j)
                g2(j - lead)
            for j in range(ncnk - lead, ncnk):
                g2(j)

    nc.compile()
    return nc


def _get_nc(nslot):
    if nslot not in _BUILD_CACHE:
        _BUILD_CACHE[nslot] = _build(nslot)
    return _BUILD_CACHE[nslot]


def _route(inputs):
    """Host gate: top-2 routing, gate weights, per-expert dispatch."""
    x = np.asarray(inputs["x"], dtype=np.float32).reshape(T, C)
    Wg = np.asarray(inputs["Wg"], dtype=np.float32)
    bg = np.asarray(inputs["bg"], dtype=np.float32)
    eb = np.asarray(inputs["expert_bias"], dtype=np.float32)

    logits = x @ Wg + bg + eb                          # [T, E]
    top2 = np.argsort(-logits, axis=1, kind="stable")[:, :TOPK]   # [T, 2]
    vals = np.take_along_axis(logits, top2, axis=1)
    probs = 1.0 / (1.0 + np.exp(-vals))
    g = probs / probs.sum(axis=1, keepdims=True)       # [T, 2]

    toks, pos_in_expert = [], np.zeros((E, T), dtype=np.int64)
    cnts = np.zeros(E, dtype=np.int64)
    sel = (top2[:, 0] == np.arange(E)[:, None]) | (top2[:, 1] == np.arange(E)[:, None])
    for e in range(E):
        tok_e = np.nonzero(sel[e])[0]
        cnts[e] = len(tok_e)
        pos_in_expert[e, tok_e] = np.arange(len(tok_e))
        toks.append(tok_e)
    return x, top2, g, toks, cnts, pos_in_expert


def _stage(x, inputs, toks, nslot):
    W1 = np.asarray(inputs["W1"], dtype=np.float32)
    b1 = np.asarray(inputs["b1"], dtype=np.float32)
    W2 = np.asarray(inputs["W2"], dtype=np.float32)

    in_maps = []
    for e in range(NCORES):
        xg = np.zeros((nslot, C), dtype=np.float32)
        xg[: len(toks[e])] = x[toks[e]]
        xgT = np.ascontiguousarray(
            xg.T.reshape(KC, P, nslot).transpose(1, 0, 2)
        ).astype(ml_dtypes.bfloat16)
        w1s = np.ascontiguousarray(
            W1[e].reshape(KC, P, HID).transpose(1, 0, 2)
        ).astype(ml_dtypes.bfloat16)
        w2s = np.ascontiguousarray(
            W2[e].reshape(KH, P, C_OUT).transpose(1, 0, 2)
        ).astype(ml_dtypes.bfloat16)
        b1s = np.ascontiguousarray(b1[e].reshape(KH, P).T)
        in_maps.append({"xg": xgT, "w1": w1s, "w2": w2s, "b1": b1s})
    return in_maps


def kernel(**inputs):
    from concourse.bass_utils import run_bass_kernel_spmd

    x, top2, g, toks, cnts, pos = _route(inputs)
    nslot = max(CH, int(-(-cnts.max() // 8) * 8))
    nc = _get_nc(nslot)
    in_maps = _stage(x, inputs, toks, nslot)
    res = run_bass_kernel_spmd(nc, in_maps, core_ids=list(range(NCORES)))

    # y[e] : [CT, P, nslot] -> [C_OUT, nslot]
    Y = np.stack(
        [np.asarray(res.results[e]["y"], dtype=np.float32).reshape(C_OUT, nslot)
         for e in range(NCORES)]
    )
    b2 = np.asarray(inputs["b2"], dtype=np.float32)
    tok_idx = np.arange(T)
    e0, e1 = top2[:, 0], top2[:, 1]
    p0 = pos[e0, tok_idx]
    p1 = pos[e1, tok_idx]
    out = (
        g[:, 0:1] * (Y[e0, :, p0] + b2[e0])
        + g[:, 1:2] * (Y[e1, :, p1] + b2[e1])
    )
    return out.reshape(B, M, H, W, C_OUT).astype(np.float32)
